# revision 37
# baseline (speedup 1.0000x reference)
"""GAT+LSTM fused kernel for 8 trn2 NeuronCores (v2).

Structure (per core, SPMD):
- Output depends only on GAT rows for nodes [110000, 120000) ("live" nodes).
- Edges sharded by src-range; self-loops assigned to the owner core
  (ownership = contiguous 1280-node slices, matching the ReduceScatter split).
- Node tables in DRAM (bf16): h_tbl rows [h(128)|a_src(4)|pad] keyed by
  shard-local src; ad64_tbl rows [a_dst(4)|pad] keyed by (p,b) slot of dst.
- Edge phase in 4 passes: one batched dma_gather per table per pass
  (amortizes the ~1us SWDGE fixed cost), vectorized attention math, and
  per-bucket one-hot PE matmuls (bf16) accumulating [msg|exp] into PSUM.
- Canonical pair-of-buckets schedule: buckets paired (2P, 2P+1); pair P gets
  ceil(max-over-cores count/128) chunks (identical across cores so the SPMD
  program is uniform; derived from the actual data at build time).
- ReduceScatter (f32) -> packed normalize (128-partition layout) -> node-major
  bf16 store -> single DMA-transpose -> W_ih matmul -> AllGather of gx.
- LSTM fixed point (ITERS passes) with software-pipelined emission: DVE runs
  only the cell scans back-to-back; gx-add is folded into PSUM via an
  identity matmul on PE; sigma/tanh on ACT; z/h muls on DVE (bf16 2x).
- FC folded into the last LSTM iteration.
"""
import os
import numpy as np
import ml_dtypes

import concourse.bass as bass
import concourse.bacc as bacc
import concourse.tile as tile
from concourse import mybir
from concourse.bass_utils import run_bass_kernel_spmd
from concourse.masks import make_identity
from contextlib import ExitStack

dt = mybir.dt
F32 = dt.float32
BF16 = dt.bfloat16
I16 = dt.int16
AF = mybir.ActivationFunctionType
ALU = mybir.AluOpType
np_bf16 = ml_dtypes.bfloat16

T, N, F_IN = 12, 10000, 64
HEADS, C, HID = 4, 32, 32
E, TN = 1_000_000, 120_000
NCORES = 8
NSH = TN // NCORES              # 15000 src-shard nodes per core
NBLK = 118                      # main node-table blocks (118*128 = 15104)
NSHP = NBLK * 128
D0 = (T - 1) * N
DN = N
DNP = 10240                     # padded live nodes
DBLK = DNP // 128               # 80 buckets
NPAIR = DBLK // 2               # 40 bucket pairs
OWN = DNP // NCORES             # 1280 owned live nodes per core
SBLK = OWN // 128               # 10 self-table blocks
NROWS = NSHP + OWN              # 16384 rows in h_tbl
HW_ROW = 256                    # h_tbl row width (bf16; 512B, %256)
AD_ROW = 128                    # ad64_tbl row width (bf16; 256B)
NPASS = 4
EAW = 7816                      # full-edge_attr wrap columns (1M vals)
LEAK = 0.2
ITERS = 4
NT = DNP
SC = 2048
NSC = NT // SC

_CACHE = {}


def _wrap16(idx):
    """int16 gather-index layout: j -> [j%16, j//16], tiled to 128 rows."""
    idx = np.asarray(idx)
    assert len(idx) % 16 == 0
    out = np.zeros((16, len(idx) // 16), np.int16)
    j = np.arange(len(idx))
    out[j % 16, j // 16] = idx.astype(np.int16)
    return np.tile(out, (8, 1))


def _chunkify(vals, cap, fill, dtype=np.float32):
    out = np.full(cap, fill, dtype)
    out[:len(vals)] = vals
    return np.ascontiguousarray(out.reshape(cap // 128, 128).T)


def _row_main(j):
    return (j % 128) * NBLK + j // 128


def _row_self(c):
    return NSHP + (c % 128) * SBLK + c // 128


def _schedule(inputs):
    """Canonical (SPMD-uniform) pair schedule from the actual edge data."""
    src = np.asarray(inputs["edge_index"][0], np.int64)
    dst = np.asarray(inputs["edge_index"][1], np.int64)
    live = (dst >= D0) & (dst < D0 + N)
    sl, dl = src[live], dst[live] - D0
    core_of = sl // NSH
    cnt = np.zeros((NCORES, NPAIR), np.int64)
    np.add.at(cnt, (core_of, ((dl % OWN) // 16) // 2), 1)
    ms = np.arange(N)
    np.add.at(cnt, (ms // OWN, ((ms % OWN) // 16) // 2), 1)
    nP = np.maximum(np.ceil(cnt.max(0) / 128).astype(int), 2)
    cstart = np.concatenate([[0], np.cumsum(nP)])   # chunk offset per pair
    nch = int(cstart[-1])
    # pass boundaries: split pairs into NPASS groups of NPAIR//NPASS
    ppp = NPAIR // NPASS
    pass_pairs = [(p * ppp, (p + 1) * ppp) for p in range(NPASS)]
    pass_chunks = [(int(cstart[a]), int(cstart[b])) for a, b in pass_pairs]
    pair_of_chunk = []
    for P in range(NPAIR):
        pair_of_chunk += [P] * int(nP[P])
    return {"nP": nP.tolist(), "cstart": cstart.tolist(), "NCH": nch,
            "pass_pairs": pass_pairs, "pass_chunks": pass_chunks,
            "pair_of_chunk": pair_of_chunk,
            "live": live, "sl": sl, "dl": dl, "core_of": core_of}


def _prep_host(inputs, sched):
    x = np.ascontiguousarray(np.asarray(inputs["x_seq"], np.float32).reshape(TN, F_IN))
    ea = np.asarray(inputs["edge_attr"], np.float32)[:, 0]
    W_gat = np.asarray(inputs["W_gat"], np.float32)
    att_src = np.asarray(inputs["att_src"], np.float32)
    att_dst = np.asarray(inputs["att_dst"], np.float32)
    att_edge = np.asarray(inputs["att_edge"], np.float32)
    W_edge = np.asarray(inputs["W_edge"], np.float32)
    gat_bias = np.asarray(inputs["gat_bias"], np.float32)
    W_ih = np.asarray(inputs["W_ih"], np.float32)
    W_hh = np.asarray(inputs["W_hh"], np.float32)
    b = np.asarray(inputs["b_ih"], np.float32) + np.asarray(inputs["b_hh"], np.float32)
    W_fc = np.asarray(inputs["W_fc"], np.float32)
    b_fc = np.asarray(inputs["b_fc"], np.float32)

    A_src = np.zeros((HEADS * C, HEADS), np.float32)
    A_dst = np.zeros((HEADS * C, HEADS), np.float32)
    for h in range(HEADS):
        A_src[h * C:(h + 1) * C, h] = att_src[h]
        A_dst[h * C:(h + 1) * C, h] = att_dst[h]
    Wa8 = np.concatenate([W_gat @ A_src, W_gat @ A_dst], axis=1)
    kap = np.array([np.dot(W_edge[0, h * C:(h + 1) * C], att_edge[h])
                    for h in range(HEADS)], np.float32)
    kap_rep = np.broadcast_to(kap, (128, HEADS)).astype(np_bf16).copy()
    gbias_rep = np.broadcast_to(gat_bias, (128, HEADS * C)).copy()
    iota128 = np.broadcast_to(np.arange(128, dtype=np.float32),
                              (128, 128)).astype(np_bf16).copy()
    c128 = np.full((128, 1), 128.0, np_bf16)
    pcol = np.arange(128, dtype=np.float32)[:, None]
    iotaP2 = np.concatenate([np.broadcast_to(pcol, (128, 128)),
                             np.broadcast_to(pcol + 128.0, (128, 128))],
                            axis=1).astype(np_bf16).copy()
    ones1 = np.ones((1, 128), np_bf16)
    perm = np.concatenate([np.arange(32, 64), np.arange(0, 32),
                           np.arange(96, 128), np.arange(64, 96)])
    WihT = np.ascontiguousarray(W_ih[perm].T.astype(np_bf16))
    WhhT = np.ascontiguousarray(W_hh[perm].T.astype(np_bf16))
    br = np.ascontiguousarray(b[perm].reshape(128, 1))
    xTD_f = x[D0:D0 + N].T                           # [64, 10000]
    xTD = np.zeros((F_IN, DNP), np_bf16)
    xTD[:, :N] = xTD_f.astype(np_bf16)

    nP, cstart, NCH = sched["nP"], sched["cstart"], sched["NCH"]
    CAP = NCH * 128
    sl, dl, core_of, live = sched["sl"], sched["dl"], sched["core_of"], sched["live"]
    ea_live = ea[live]
    ea_all = ea.reshape(NCORES, E // NCORES)

    in_maps = []
    for k in range(NCORES):
        m = core_of == k
        sLk = sl[m] - k * NSH                        # shard-local src
        dLk = dl[m]
        eak = ea_live[m]
        own_lo = OWN * k
        selfn = np.arange(own_lo, min(own_lo + OWN, N))
        # combined edge list: regular then self
        e_hs = _row_main(sLk)
        s_hs = _row_self(selfn - own_lo)
        allm = np.concatenate([dLk, selfn])
        hsrow = np.concatenate([e_hs, s_hs])
        eav = np.concatenate([eak, np.zeros(len(selfn), np.float32)])
        selfm = np.concatenate([np.zeros(len(dLk), np.float32),
                                np.ones(len(selfn), np.float32)])
        p_of = 16 * (allm // OWN) + (allm % 16)
        b_of = (allm % OWN) // 16
        adrow = p_of * DBLK + b_of
        P_of = b_of // 2
        sv = 128 * (b_of % 2) + p_of
        # pack per pair
        order = np.argsort(P_of, kind="stable")
        hs_idx = np.zeros(CAP, np.int64)
        ad_idx = np.zeros(CAP, np.int64)
        eac = np.zeros(CAP, np.float32)
        svf = np.full(CAP, -1.0, np.float32)
        smf = np.zeros(CAP, np.float32)
        Psort = P_of[order]
        bounds = np.searchsorted(Psort, np.arange(NPAIR + 1))
        for P in range(NPAIR):
            sel = order[bounds[P]:bounds[P + 1]]
            nb = len(sel)
            capP = nP[P] * 128
            assert nb <= capP, f"core {k} pair {P}: {nb} > {capP}"
            o = cstart[P] * 128
            hs_idx[o:o + nb] = hsrow[sel]
            ad_idx[o:o + nb] = adrow[sel]
            eac[o:o + nb] = eav[sel]
            svf[o:o + nb] = sv[sel]
            smf[o:o + nb] = selfm[sel]
        # per-pass wrapped int16 gather indices
        hsI = np.concatenate(
            [_wrap16(hs_idx[c0 * 128:c1 * 128]) for c0, c1 in sched["pass_chunks"]],
            axis=1)
        svT = np.ascontiguousarray(svf.reshape(1, CAP).astype(np_bf16))
        xT = np.zeros((F_IN, NSHP), np_bf16)
        xT[:, :NSH] = x[k * NSH:(k + 1) * NSH].T.astype(np_bf16)
        # xTD columns permuted to (b, p) slot order for the a_dst table matmuls
        # column b*128 + p holds node m = OWN*(p//16) + 16*b + p%16
        pq = np.arange(128)
        cols = np.empty((DBLK, 128), np.int64)
        for bb in range(DBLK):
            cols[bb] = OWN * (pq // 16) + 16 * bb + (pq % 16)
        xTDP = np.ascontiguousarray(xTD[:, cols.ravel()])
        xTDS = np.ascontiguousarray(xTD[:, own_lo:own_lo + OWN])
        eaF = np.zeros((128, 980), np.float32)
        ch = ea_all[k]
        j2 = np.arange(len(ch))
        eaF[j2 % 128, j2 // 128] = ch
        in_maps.append({
            "xT": xT, "xTDP": xTDP, "xTDS": xTDS,
            "eaC": _chunkify(eac, CAP, 0.0, np_bf16),
            "svF": _chunkify(svf, CAP, -1.0, np_bf16),
            "svH": _chunkify(svf - 128.0, CAP, -129.0, np_bf16),
            "selfM": _chunkify(smf, CAP, 0.0, np_bf16),
            "eaF": eaF, "hsI": hsI, "svT": svT,
            "Wgat": W_gat.astype(np_bf16), "Wa8": Wa8.astype(np_bf16),
            "kap": kap_rep, "gbias": gbias_rep, "iota": iota128, "c128": c128,
            "iotaP2": iotaP2, "ones1": ones1,
            "Wih": WihT, "Whh": WhhT, "br": br,
            "Wfc": np.ascontiguousarray(W_fc.reshape(HID, 1).astype(np_bf16)),
            "bfc": np.ascontiguousarray(b_fc.reshape(1, 1)),
        })
    return in_maps


def _build_nc(sched, debug=False):
    STAGE = int(os.environ.get("KSTAGE", "99"))
    nP, cstart, NCH = sched["nP"], sched["cstart"], sched["NCH"]
    pass_pairs, pass_chunks = sched["pass_pairs"], sched["pass_chunks"]
    pair_of_chunk = sched["pair_of_chunk"]
    nc = bacc.Bacc("TRN2", target_bir_lowering=False, debug=False,
                   num_devices=NCORES, num_swdge_queues=2)
    g = lambda n, s, d=F32: nc.dram_tensor(n, s, d, kind="ExternalInput").ap()
    xT = g("xT", [F_IN, NSHP], BF16)
    xTDP = g("xTDP", [F_IN, DNP], BF16)
    xTDS = g("xTDS", [F_IN, OWN], BF16)
    eaC = g("eaC", [128, NCH], BF16); svF = g("svF", [128, NCH], BF16)
    svH = g("svH", [128, NCH], BF16)
    selfM = g("selfM", [128, NCH], BF16); eaF = g("eaF", [128, 980])
    hsI = g("hsI", [128, NCH * 8], I16); svT = g("svT", [1, NCH * 128], BF16)
    Wgat = g("Wgat", [F_IN, 128], BF16); Wa8 = g("Wa8", [F_IN, 8], BF16)
    kap = g("kap", [128, HEADS], BF16); gbias = g("gbias", [128, 128])
    iota = g("iota", [128, 128], BF16); c128 = g("c128", [128, 1], BF16)
    iotaP2 = g("iotaP2", [128, 256], BF16); ones1 = g("ones1", [1, 128], BF16)
    Wih = g("Wih", [128, 128], BF16); Whh = g("Whh", [HID, 128], BF16)
    br = g("br", [128, 1]); Wfc = g("Wfc", [HID, 1], BF16); bfc = g("bfc", [1, 1])
    out = nc.dram_tensor("out", [1, NT], F32, kind="ExternalOutput").ap()
    if debug:
        dbg_gat = nc.dram_tensor("dbg_gat", [128, DNP], F32, kind="ExternalOutput").ap()
        dbg_acc = nc.dram_tensor("dbg_acc", [128, DBLK * 132], F32,
                                 kind="ExternalOutput").ap()
        dbg_rs = nc.dram_tensor("dbg_rs", [16, DBLK * 132], F32,
                                kind="ExternalOutput").ap()
        dbg_nd = nc.dram_tensor("dbg_nd", [OWN, 132], F32,
                                kind="ExternalOutput").ap()
        dbg_np = nc.dram_tensor("dbg_np", [128, (DBLK // 8) * 132], F32,
                                kind="ExternalOutput").ap()
        dbg_gx = nc.dram_tensor("dbg_gx", [128, NT], F32, kind="ExternalOutput").ap()
        dbg_h = nc.dram_tensor("dbg_h", [HID, NT], F32, kind="ExternalOutput").ap()

    h_tbl = nc.dram_tensor("h_tbl", [NROWS, HW_ROW], BF16).ap()
    acc_tbl = nc.dram_tensor("acc_tbl", [128, DBLK * 132], F32).ap()
    rs_out = nc.dram_tensor("rs_out", [16, DBLK * 132], F32).ap()
    gat_nd = nc.dram_tensor("gat_nd", [OWN, 132], BF16).ap()
    gx_sh = nc.dram_tensor("gx_sh", [128, OWN], BF16).ap()
    gx_full = nc.dram_tensor("gx_full", [NCORES, 128, OWN], BF16,
                             addr_space="Shared").ap()
    ea_in = nc.dram_tensor("ea_in", [128, 1], F32).ap()
    ea_out = nc.dram_tensor("ea_out", [128, 1], F32, addr_space="Shared").ap()
    RG = [list(range(NCORES))]

    def ecopy(e, dst, src):
        if e is nc.scalar:
            e.copy(dst, src)
        else:
            e.tensor_copy(dst, src)

    with tile.TileContext(nc) as tc, ExitStack() as top:
        const = top.enter_context(tc.tile_pool(name="const", bufs=1))
        ident = const.tile([128, 128], F32)
        make_identity(nc, ident[:])
        identb = const.tile([128, 128], BF16)
        nc.vector.tensor_copy(identb[:], ident[:])
        wgat_t = const.tile([F_IN, 128], BF16); nc.sync.dma_start(wgat_t[:], Wgat[:])
        wa8_t = const.tile([F_IN, 8], BF16); nc.sync.dma_start(wa8_t[:], Wa8[:])
        kap_t = const.tile([128, HEADS], BF16); nc.sync.dma_start(kap_t[:], kap[:])
        gb_t = const.tile([128, 128], F32); nc.sync.dma_start(gb_t[:], gbias[:])
        iota_t = const.tile([128, 128], BF16); nc.sync.dma_start(iota_t[:], iota[:])
        c128_t = const.tile([128, 1], BF16); nc.sync.dma_start(c128_t[:], c128[:])
        whh_t = const.tile([HID, 128], BF16); nc.sync.dma_start(whh_t[:], Whh[:])
        wih_t = const.tile([128, 128], BF16); nc.sync.dma_start(wih_t[:], Wih[:])
        br_t = const.tile([128, 1], F32); nc.sync.dma_start(br_t[:], br[:])
        wfc_t = const.tile([HID, 1], BF16); nc.sync.dma_start(wfc_t[:], Wfc[:])
        bfc_t = const.tile([1, 1], F32); nc.sync.dma_start(bfc_t[:], bfc[:])
        iop2_t = const.tile([128, 256], BF16); nc.sync.dma_start(iop2_t[:], iotaP2[:])
        on1_t = const.tile([1, 128], BF16); nc.sync.dma_start(on1_t[:], ones1[:])
        meanr = const.tile([128, 1], F32)
        adt_t = const.tile([128, DBLK * 4], BF16)
        sched_p = top.enter_context(tc.tile_pool(name="sched", bufs=1))
        eaC_t = sched_p.tile([128, NCH], BF16)
        nc.sync.dma_start(eaC_t[:], eaC[:])
        svF_t = sched_p.tile([128, NCH], BF16)
        nc.sync.dma_start(svF_t[:], svF[:])
        svH_t = sched_p.tile([128, NCH], BF16)
        nc.sync.dma_start(svH_t[:], svH[:])
        sm_t = sched_p.tile([128, NCH], BF16)
        nc.sync.dma_start(sm_t[:], selfM[:])
        hsI_t = sched_p.tile([128, NCH * 8], I16)
        nc.sync.dma_start(hsI_t[:], hsI[:])

        # ---------- head: load x slices up-front, A1 mean, node tables ------
        with ExitStack() as ph:
            sbh = ph.enter_context(tc.tile_pool(name="sbh", bufs=1))
            psa = ph.enter_context(tc.tile_pool(name="psa", bufs=3, space="PSUM"))
            psp = ph.enter_context(tc.tile_pool(name="psp", bufs=1, space="PSUM"))
            xt_t = sbh.tile([F_IN, NSHP], BF16)
            nc.sync.dma_start(xt_t[:], xT[:])
            xtd_t = sbh.tile([F_IN, DNP], BF16)
            nc.sync.dma_start(xtd_t[:], xTDP[:])
            xts_t = sbh.tile([F_IN, OWN], BF16)
            nc.sync.dma_start(xts_t[:], xTDS[:])

            # A1: mean(edge_attr) via AllReduce + PE broadcast; the early
            # collective doubles as a launch-skew barrier so the later
            # ReduceScatter doesn't absorb cross-core skew on the critical path
            with ExitStack() as p1:
                sbm = p1.enter_context(tc.tile_pool(name="sbm", bufs=1))
                psm = p1.enter_context(tc.tile_pool(name="psm", bufs=1,
                                                    space="PSUM"))
                eaf_t = sbm.tile([128, 980], F32)
                nc.sync.dma_start(eaf_t[:], eaF[:])
                eap = sbm.tile([128, 1], F32)
                nc.vector.tensor_reduce(eap[:], eaf_t[:], mybir.AxisListType.X,
                                        ALU.add)
                nc.sync.dma_start(ea_in[:], eap[:])
                nc.gpsimd.collective_compute("AllReduce", ALU.add,
                                             replica_groups=RG,
                                             ins=[ea_in[:]], outs=[ea_out[:]])
                eao_t = sbm.tile([128, 1], F32)
                nc.sync.dma_start(eao_t[:], ea_out[:])
                onc = sbm.tile([128, 1], F32)
                nc.gpsimd.memset(onc[:], 1.0)
                ps1 = psm.tile([1, 1], F32, space="PSUM", tag="ps1")
                nc.tensor.matmul(ps1[:], lhsT=eao_t[:], rhs=onc[:], start=True,
                                 stop=True)
                eas = sbm.tile([1, 1], F32)
                nc.scalar.mul(eas[:], ps1[:], 1.0 / E)
                onr = sbm.tile([1, 128], F32)
                nc.gpsimd.memset(onr[:], 1.0)
                ps2 = psm.tile([128, 1], F32, space="PSUM", tag="ps2")
                nc.tensor.matmul(ps2[:], lhsT=onr[:], rhs=eas[:], start=True,
                                 stop=True)
                nc.vector.tensor_copy(meanr[:], ps2[:])

            if STAGE >= 1:
                # A3: a_dst values resident in SBUF [p, b*4+h]
                pack_d = psp.tile([128, DBLK * 4], F32, space="PSUM", tag="pkd")
                for bb in range(DBLK):
                    nc.tensor.matmul(pack_d[:, bb * 4:(bb + 1) * 4],
                                     lhsT=xtd_t[:, bb * 128:(bb + 1) * 128],
                                     rhs=wa8_t[:, 4:8], start=True, stop=True)
                nc.vector.tensor_copy(adt_t[:], pack_d[:])

                # A2: main h table (118 blocks)
                HS = sbh.tile([128, NBLK * HW_ROW], BF16)
                HSv = HS[:].rearrange("p (j w) -> p j w", w=HW_ROW)
                nc.gpsimd.memset(HSv[:, :, 132:HW_ROW], 0.0)
                pack_a = psp.tile([128, NBLK * 4], F32, space="PSUM", tag="pka")
                eng = [nc.vector, nc.scalar]
                for j in range(NBLK):
                    ph_ = psa.tile([128, 128], F32, space="PSUM", tag="ph")
                    nc.tensor.matmul(ph_[:], lhsT=xt_t[:, j * 128:(j + 1) * 128],
                                     rhs=wgat_t[:], start=True, stop=True)
                    ecopy(eng[j % 2], HSv[:, j, 0:128], ph_[:])
                    nc.tensor.matmul(pack_a[:, j * 4:(j + 1) * 4],
                                     lhsT=xt_t[:, j * 128:(j + 1) * 128],
                                     rhs=wa8_t[:, 0:4], start=True, stop=True)
                nc.vector.tensor_copy(HSv[:, :, 128:132],
                                      pack_a[:].rearrange("p (j w) -> p j w", w=4))
                # store in 4 groups so DMA overlaps the matmul tail
                h_main = h_tbl[0:NSHP, :].rearrange("(p j) w -> p (j w)", p=128)
                gs = NBLK // 4 + 1
                for gi in range(4):
                    lo = gi * gs * HW_ROW
                    hi = min((gi + 1) * gs, NBLK) * HW_ROW
                    if lo < hi:
                        nc.sync.dma_start(h_main[:, lo:hi], HS[:, lo:hi])

                # A2b: own-live self rows
                HS2 = sbh.tile([128, SBLK * HW_ROW], BF16)
                HS2v = HS2[:].rearrange("p (j w) -> p j w", w=HW_ROW)
                nc.gpsimd.memset(HS2v[:, :, 132:HW_ROW], 0.0)
                pack_s = psp.tile([128, SBLK * 4], F32, space="PSUM", tag="pks")
                for j in range(SBLK):
                    ph2 = psa.tile([128, 128], F32, space="PSUM", tag="ph")
                    nc.tensor.matmul(ph2[:], lhsT=xts_t[:, j * 128:(j + 1) * 128],
                                     rhs=wgat_t[:], start=True, stop=True)
                    ecopy(eng[j % 2], HS2v[:, j, 0:128], ph2[:])
                    nc.tensor.matmul(pack_s[:, j * 4:(j + 1) * 4],
                                     lhsT=xts_t[:, j * 128:(j + 1) * 128],
                                     rhs=wa8_t[:, 0:4], start=True, stop=True)
                nc.vector.tensor_copy(HS2v[:, :, 128:132],
                                      pack_s[:].rearrange("p (j w) -> p j w", w=4))
                nc.sync.dma_start(
                    h_tbl[NSHP:NROWS, :].rearrange("(p j) w -> p (j w)", p=128),
                    HS2[:])

        if STAGE >= 2:
            # ---------- edge phase: 4 passes, batched gathers ----------
            with ExitStack() as ph:
                sbe = ph.enter_context(tc.tile_pool(name="sbe", bufs=1))
                sbp = ph.enter_context(tc.tile_pool(name="sbp", bufs=2))
                sbq = ph.enter_context(tc.tile_pool(name="sbq", bufs=2))
                pse = ph.enter_context(tc.tile_pool(name="pse", bufs=2,
                                                    space="PSUM"))
                ACCT = sbe.tile([128, DBLK * 132], F32)
                ceng = [nc.scalar, nc.vector, nc.scalar]
                cn = 0
                psr = ph.enter_context(tc.tile_pool(name="psr", bufs=2,
                                                    space="PSUM"))
                psA = ph.enter_context(tc.tile_pool(name="psA", bufs=2,
                                                    space="PSUM"))
                for p in range(NPASS):
                    c0, c1 = pass_chunks[p]
                    PC = c1 - c0
                    NIDX = PC * 128
                    NH = sbp.tile([128, PC * HW_ROW], BF16, tag="NH")
                    NHB = sbp.tile([128, PC * 132], BF16, tag="NHB")
                    NHv = NH[:].rearrange("p (e w) -> p e w", w=HW_ROW)
                    NHBv = NHB[:].rearrange("p (e w) -> p e w", w=132)
                    nc.gpsimd.dma_gather(
                        out_ap=NHv, in_ap=h_tbl[:],
                        idxs_ap=hsI_t[:, c0 * 8:c1 * 8],
                        num_idxs=NIDX, num_idxs_reg=NIDX, elem_size=HW_ROW,
                        single_packet=False, queue_num=p % 2)
                    # a_dst per slot: replicate sv down partitions (PE), build
                    # transposed one-hots, then 4-col matmuls vs resident adt_t
                    svT_p = sbq.tile([1, PC * 128], BF16, tag="svT")
                    nc.sync.dma_start(svT_p[:], svT[0:1, c0 * 128:c1 * 128])
                    AD4ps = psA.tile([128, PC * 4], F32, space="PSUM", tag="AD4")
                    for gch in range(0, PC, 4):
                        ng = min(4, PC - gch)
                        SR = psr.tile([128, 512], F32, space="PSUM", tag="SR")
                        nc.tensor.matmul(SR[:, 0:ng * 128], lhsT=on1_t[:],
                                         rhs=svT_p[:, gch * 128:(gch + ng) * 128],
                                         start=True, stop=True)
                        OT = sbq.tile([128, 1024], BF16, tag="OT")
                        OTv = OT[:].rearrange("p (l e w) -> p l e w", l=2, w=128)
                        srv = SR[:, 0:ng * 128] \
                            .rearrange("p (o e w) -> p o e w", o=1, w=128) \
                            .to_broadcast([128, 2, ng, 128])
                        iov = iop2_t[:].rearrange("p (l o w) -> p l o w",
                                                  l=2, o=1) \
                            .to_broadcast([128, 2, ng, 128])
                        nc.vector.tensor_tensor(out=OTv[:, :, 0:ng, :], in0=srv,
                                                in1=iov, op=ALU.is_equal)
                        for j in range(ng):
                            lc = gch + j
                            Pj = pair_of_chunk[c0 + lc]
                            nc.tensor.matmul(
                                AD4ps[:, lc * 4:(lc + 1) * 4],
                                lhsT=OTv[:, 0, j, :],
                                rhs=adt_t[:, (2 * Pj) * 4:(2 * Pj + 1) * 4],
                                start=True, stop=False)
                            nc.tensor.matmul(
                                AD4ps[:, lc * 4:(lc + 1) * 4],
                                lhsT=OTv[:, 1, j, :],
                                rhs=adt_t[:, (2 * Pj + 1) * 4:(2 * Pj + 2) * 4],
                                start=False, stop=True)
                    AD4t = sbq.tile([128, PC * 4], BF16, tag="AD4t")
                    nc.vector.tensor_copy(AD4t[:], AD4ps[:])
                    # q = a_src + a_dst + (ea + selfM*mean) * kap ; leaky; exp
                    EAm = sbq.tile([128, PC], F32, tag="EAm")
                    nc.vector.scalar_tensor_tensor(
                        out=EAm[:], in0=sm_t[:, c0:c1], scalar=meanr[:],
                        op0=ALU.mult, op1=ALU.add, in1=eaC_t[:, c0:c1])
                    Q4 = sbq.tile([128, PC * 4], F32, tag="Q4")
                    Q4v = Q4[:].rearrange("p (e w) -> p e w", w=4)
                    nc.vector.tensor_tensor(out=Q4v, in0=NHv[:, :, 128:132],
                                            in1=AD4t[:].rearrange(
                                                "p (e w) -> p e w", w=4),
                                            op=ALU.add)
                    T2 = sbq.tile([128, PC * 4], F32, tag="T2")
                    T2v = T2[:].rearrange("p (e w) -> p e w", w=4)
                    ea3 = EAm[:].rearrange("p (e w) -> p e w", w=1) \
                        .to_broadcast([128, PC, 4])
                    kap3 = kap_t[:].rearrange("p (o w) -> p o w", o=1) \
                        .to_broadcast([128, PC, 4])
                    nc.vector.tensor_tensor(out=T2v, in0=ea3, in1=kap3, op=ALU.mult)
                    nc.vector.tensor_tensor(out=Q4v, in0=Q4v, in1=T2v, op=ALU.add)
                    nc.vector.tensor_scalar_mul(T2[:], Q4[:], LEAK)
                    nc.vector.tensor_tensor(out=Q4[:], in0=Q4[:], in1=T2[:],
                                            op=ALU.max)
                    # exp(q) = sigmoid(q)/sigmoid(-q)
                    SG1 = sbq.tile([128, PC * 4], F32, tag="SG1")
                    nc.scalar.activation(SG1[:], Q4[:], AF.Sigmoid)
                    S4 = sbq.tile([128, PC * 4], F32, tag="S4")
                    nc.scalar.activation(S4[:], Q4[:], AF.Sigmoid, scale=-1.0)
                    nc.vector.reciprocal(S4[:], S4[:])
                    nc.vector.tensor_tensor(out=S4[:], in0=SG1[:], in1=S4[:],
                                            op=ALU.mult)
                    E4b = sbq.tile([128, PC * 4], BF16, tag="E4b")
                    nc.scalar.copy(E4b[:], S4[:])
                    S4v = S4[:].rearrange("p (e w) -> p e w", w=4)
                    nc.scalar.copy(NHBv[:, :, 128:132], S4v)
                    # expand exp to full width (bf16), then packed 2x multiply
                    E4X = sbp.tile([128, PC * 128], BF16, tag="E4X")
                    e4xv = E4X[:].rearrange("p (e h c) -> p e h c", h=HEADS, c=C)
                    e4b = E4b[:].rearrange("p (e h c) -> p e h c", h=HEADS, c=1) \
                        .to_broadcast([128, PC, HEADS, C])
                    nc.scalar.copy(e4xv, e4b)
                    nc.vector.tensor_tensor(
                        out=NHBv[:, :, 0:128], in0=NHv[:, :, 0:128],
                        in1=E4X[:].rearrange("p (e w) -> p e w", w=128),
                        op=ALU.mult)
                    # one-hots for the whole pass (bf16, 2x DVE)
                    OHL = sbp.tile([128, PC * 128], BF16, tag="OHL")
                    OHH = sbp.tile([128, PC * 128], BF16, tag="OHH")
                    OHLv = OHL[:].rearrange("p (e w) -> p e w", w=128)
                    OHHv = OHH[:].rearrange("p (e w) -> p e w", w=128)
                    sv3 = svF_t[:, c0:c1].rearrange("p (e w) -> p e w", w=1) \
                        .to_broadcast([128, PC, 128])
                    io3 = iota_t[:].rearrange("p (o w) -> p o w", o=1) \
                        .to_broadcast([128, PC, 128])
                    nc.vector.tensor_tensor(out=OHLv, in0=sv3, in1=io3,
                                            op=ALU.is_equal)
                    svh3 = svH_t[:, c0:c1].rearrange("p (e w) -> p e w", w=1) \
                        .to_broadcast([128, PC, 128])
                    nc.vector.tensor_tensor(out=OHHv, in0=svh3, in1=io3,
                                            op=ALU.is_equal)
                    for P in range(*pass_pairs[p]):
                        npc = nP[P]
                        lc = cstart[P] - c0
                        plo = pse.tile([128, 132], F32, space="PSUM", tag="plo")
                        phi = pse.tile([128, 132], F32, space="PSUM", tag="phi")
                        for j in range(npc):
                            nc.tensor.matmul(plo[:], lhsT=OHLv[:, lc + j, :],
                                             rhs=NHBv[:, lc + j, 0:132],
                                             start=(j == 0), stop=(j == npc - 1))
                            nc.tensor.matmul(phi[:], lhsT=OHHv[:, lc + j, :],
                                             rhs=NHBv[:, lc + j, 0:132],
                                             start=(j == 0), stop=(j == npc - 1))
                        ecopy(ceng[cn % 3],
                              ACCT[:, (2 * P) * 132:(2 * P + 1) * 132], plo[:])
                        cn += 1
                        ecopy(ceng[cn % 3],
                              ACCT[:, (2 * P + 1) * 132:(2 * P + 2) * 132], phi[:])
                        cn += 1
                if STAGE >= 3:
                    nc.sync.dma_start(acc_tbl[:], ACCT[:])
                    nc.gpsimd.collective_compute(
                        "ReduceScatter", ALU.add, replica_groups=RG,
                        ins=[acc_tbl[:]], outs=[rs_out[:]])
                if debug:
                    nc.sync.dma_start(dbg_acc[:], ACCT[:])

        if STAGE >= 3:
            # ---------- packed normalize + transpose --------
            with ExitStack() as ph:
                sbn = ph.enter_context(tc.tile_pool(name="sbn", bufs=1))
                NP_ = sbn.tile([128, (DBLK // 8) * 132], F32)
                # packed load: partition p' = 16*jg + p, jg = j // 10
                RW = (DBLK // 8) * 132
                for jg in range(8):
                    nc.sync.dma_start(NP_[16 * jg:16 * (jg + 1), :],
                                      rs_out[:, jg * RW:(jg + 1) * RW])
                if debug:
                    nc.sync.dma_start(dbg_np[:], NP_[:])
                JW = DBLK // 8           # 10 buckets per partition group
                NPv = NP_[:].rearrange("p (j w) -> p j w", w=132)
                nc.vector.tensor_scalar_add(NPv[:, :, 128:132],
                                            NPv[:, :, 128:132], 1e-16)
                RC = sbn.tile([128, JW * 4], F32)
                RCv = RC[:].rearrange("p (j w) -> p j w", w=4)
                nc.vector.reciprocal(RCv, NPv[:, :, 128:132])
                r4 = RCv.rearrange("p j (h c) -> p j h c", c=1) \
                    .to_broadcast([128, JW, HEADS, C])
                m4 = NPv[:, :, 0:128].rearrange("p j (h c) -> p j h c", h=HEADS)
                nc.vector.tensor_tensor(out=m4, in0=m4, in1=r4, op=ALU.mult)
                gbb = gb_t[:].rearrange("p (o w) -> p o w", o=1) \
                    .to_broadcast([128, JW, 128])
                nc.vector.tensor_tensor(out=NPv[:, :, 0:128], in0=NPv[:, :, 0:128],
                                        in1=gbb, op=ALU.add)
                nc.vector.tensor_scalar_max(NPv[:, :, 0:128], NPv[:, :, 0:128], 0.0)
                NB = sbn.tile([128, JW * 132], BF16)
                nc.vector.tensor_copy(NB[:], NP_[:])
                # node-major store: node = 160*jg + 16*jj + p (8 3D DMAs)
                for jg in range(8):
                    ndv = gat_nd[160 * jg:160 * (jg + 1), :] \
                        .rearrange("(jj p) w -> p jj w", p=16)
                    nbv = NB[16 * jg:16 * (jg + 1), :] \
                        .rearrange("p (jj w) -> p jj w", w=132)
                    nc.sync.dma_start(ndv, nbv)
                if debug:
                    nc.sync.dma_start(dbg_rs[:], rs_out[:])
                    nc.gpsimd.dma_start(dbg_nd[:], gat_nd[:])
        if STAGE >= 4:
            # ---------- transpose-load + W_ih matmul + AllGather ----------
            with ExitStack() as ph:
                sbg = ph.enter_context(tc.tile_pool(name="sbg", bufs=1))
                psg = ph.enter_context(tc.tile_pool(name="psg", bufs=3,
                                                    space="PSUM"))
                GBt = sbg.tile([128, OWN], BF16)
                nc.sync.dma_start_transpose(GBt[:], gat_nd[:, 0:128])
                gxs = sbg.tile([128, OWN], BF16)
                for q in range(OWN // 512 + (1 if OWN % 512 else 0)):
                    lo = q * 512
                    hi = min(lo + 512, OWN)
                    pg = psg.tile([128, 512], F32, space="PSUM", tag="pg")
                    nc.tensor.matmul(pg[:, 0:hi - lo], lhsT=wih_t[:],
                                     rhs=GBt[:, lo:hi], start=True, stop=True)
                    nc.vector.tensor_scalar_add(gxs[:, lo:hi], pg[:, 0:hi - lo],
                                                br_t[:])
                nc.sync.dma_start(gx_sh[:], gxs[:])
            nc.gpsimd.collective_compute("AllGather", ALU.bypass,
                                         replica_groups=RG,
                                         ins=[gx_sh[:]], outs=[gx_full[:]])

        # ---------- LSTM fixed point (software-pipelined emission) ----------
        persist = top.enter_context(tc.tile_pool(name="persist", bufs=1))
        gxt = persist.tile([128, NT], BF16)
        H = persist.tile([HID, NT + 32], BF16)
        nc.gpsimd.memset(H[:], 0.0)
        if STAGE >= 5:
            nc.sync.dma_start(gxt[:].rearrange("p (k n) -> p k n", k=NCORES),
                              gx_full[:].rearrange("k p n -> p k n"))
        else:
            nc.gpsimd.memset(gxt[:], 0.0)
        if debug:
            nc.gpsimd.dma_start(dbg_gx[:], gxt[:])
        if STAGE >= 6:
            with ExitStack() as ph:
                sbl = ph.enter_context(tc.tile_pool(name="sbl", bufs=7))
                sbc = ph.enter_context(tc.tile_pool(name="sbc", bufs=3))
                sbo = ph.enter_context(tc.tile_pool(name="sbo", bufs=2))
                psl = ph.enter_context(tc.tile_pool(name="psl", bufs=3,
                                                    space="PSUM"))
                psf = ph.enter_context(tc.tile_pool(name="psf", bufs=2,
                                                    space="PSUM"))
                S_t = [None] * NSC
                Tg_t = [None] * NSC
                Zt_t = [None] * NSC
                Ct_t = [None] * NSC

                def emit_zt(s):
                    Zt = sbl.tile([HID, SC], BF16, tag="Zt")
                    nc.gpsimd.tensor_tensor(out=Zt[:], in0=S_t[s][32:64, :],
                                            in1=Tg_t[s][32:64, :], op=ALU.mult)
                    Zt_t[s] = Zt

                def stage_in(i, s):
                    """matmuls into PSUM, activations, z — for iteration i."""
                    lo = s * SC
                    S_ = sbl.tile([96, SC], BF16, tag="S")
                    Tg = sbl.tile([64, SC], BF16, tag="Tg")
                    if i == 0:
                        nc.scalar.activation(S_[:], gxt[0:96, lo:lo + SC],
                                             AF.Sigmoid)
                        nc.scalar.activation(Tg[32:64, :], gxt[96:128, lo:lo + SC],
                                             AF.Tanh)
                    else:
                        for q in range(SC // 1024):
                            a = lo + q * 1024
                            pG = psl.tile([128, 1024], F32, space="PSUM", tag="pG")
                            for hh in range(2):
                                ha, hb = a + hh * 512, a + (hh + 1) * 512
                                nc.tensor.matmul(pG[:, hh * 512:(hh + 1) * 512],
                                                 lhsT=whh_t[:], rhs=H[:, ha:hb],
                                                 start=True, stop=False)
                                nc.tensor.matmul(pG[:, hh * 512:(hh + 1) * 512],
                                                 lhsT=identb[:], rhs=gxt[:, ha:hb],
                                                 start=False, stop=True)
                            nc.scalar.activation(S_[:, q * 1024:(q + 1) * 1024],
                                                 pG[0:96, :], AF.Sigmoid)
                            nc.scalar.activation(Tg[32:64, q * 1024:(q + 1) * 1024],
                                                 pG[96:128, :], AF.Tanh)
                    S_t[s] = S_
                    Tg_t[s] = Tg

                TC_t = [None] * NSC

                def stage_out(i, s):
                    """tanh(c) — for iteration i."""
                    TC = sbo.tile([96, SC], BF16, tag="TC")
                    nc.scalar.activation(TC[64:96, :], Ct_t[s][:], AF.Tanh)
                    TC_t[s] = TC

                def emit_hm(i, s):
                    lo = s * SC
                    nc.vector.tensor_tensor(out=H[:, lo + 1:lo + SC + 1],
                                            in0=S_t[s][64:96, :],
                                            in1=TC_t[s][64:96, :], op=ALU.mult)

                for s in range(NSC):
                    stage_in(0, s)
                    emit_zt(s)
                def fc_chunk(s):
                    lo = s * SC
                    OFc = sbo.tile([1, SC], F32, tag="OFc")
                    for q in range(SC // 512):
                        pf = psf.tile([1, 512], F32, space="PSUM", tag="pf")
                        nc.tensor.matmul(
                            pf[:], lhsT=wfc_t[:],
                            rhs=H[:, 1 + lo + q * 512:1 + lo + (q + 1) * 512],
                            start=True, stop=True)
                        nc.vector.tensor_scalar_add(
                            OFc[:, q * 512:(q + 1) * 512], pf[:], bfc_t[:])
                    nc.sync.dma_start(out[:, lo:lo + SC], OFc[:])

                for i in range(ITERS):
                    for s in range(NSC):
                        Ct = sbc.tile([HID, SC], F32, tag="Ct")
                        nc.vector.tensor_tensor_scan(
                            out=Ct[:], data0=S_t[s][0:32, :], data1=Zt_t[s][:],
                            initial=(0.0 if s == 0 else Ct_t[s - 1][:, SC - 1:SC]),
                            op0=ALU.mult, op1=ALU.add)
                        Ct_t[s] = Ct
                        stage_out(i, s)
                        # lagged emissions so DVE never stalls behind the
                        # cross-iteration tc->hm->matmul->sigmoid chain:
                        # hm and the next iteration's inputs lag 1 chunk,
                        # the next iteration's z-mult lags 3 chunks
                        if s >= 1:
                            emit_hm(i, s - 1)
                            if i == ITERS - 1:
                                fc_chunk(s - 1)
                            elif i + 1 < ITERS:
                                stage_in(i + 1, s - 1)
                                if s >= 4:
                                    emit_zt(s - 4)
                    emit_hm(i, NSC - 1)
                    if i == ITERS - 1:
                        fc_chunk(NSC - 1)
                    if i + 1 < ITERS:
                        stage_in(i + 1, NSC - 1)
                        for sz in range(max(0, NSC - 4), NSC):
                            emit_zt(sz)
        else:
            with ExitStack() as ph:
                sbf = ph.enter_context(tc.tile_pool(name="sbf", bufs=1))
                OF = sbf.tile([1, NT], F32)
                nc.gpsimd.memset(OF[:], 0.0)
                nc.sync.dma_start(out[:], OF[:])
        if debug:
            nc.gpsimd.dma_start(dbg_h[:], H[:, 1:NT + 1])
            with ExitStack() as ph:
                sbd = ph.enter_context(tc.tile_pool(name="sbd", bufs=1))
                DG = sbd.tile([128, OWN], F32)
                DB = sbd.tile([128, OWN], BF16)
                nc.sync.dma_start_transpose(DB[:], gat_nd[:, 0:128])
                nc.vector.tensor_copy(DG[:], DB[:])
                nc.sync.dma_start(dbg_gat[:, 0:OWN], DG[:])

    nc.compile()
    return nc


def run(inputs, trace=False, debug=False):
    sched = _schedule(inputs)
    key = ("dbg" if debug else "rel", sched["NCH"], tuple(sched["nP"]))
    if key not in _CACHE:
        _CACHE[key] = _build_nc(sched, debug=debug)
    nc = _CACHE[key]
    in_maps = _prep_host(inputs, sched)
    res = run_bass_kernel_spmd(nc, in_maps, list(range(NCORES)), trace=trace)
    return res


def kernel(**inputs) -> np.ndarray:
    res = run(inputs)
    o = res.results[0]["out"]
    return np.ascontiguousarray(o[0, :N].reshape(N, 1).astype(np.float32))


# revision 38
# speedup vs baseline: 1.0922x; 1.0922x over previous
"""GAT+LSTM fused kernel for 8 trn2 NeuronCores (v2).

Structure (per core, SPMD):
- Output depends only on GAT rows for nodes [110000, 120000) ("live" nodes).
- Edges sharded by src-range; self-loops assigned to the owner core
  (ownership = contiguous 1280-node slices, matching the ReduceScatter split).
- Node tables in DRAM (bf16): h_tbl rows [h(128)|a_src(4)|pad] keyed by
  shard-local src; ad64_tbl rows [a_dst(4)|pad] keyed by (p,b) slot of dst.
- Edge phase in 4 passes: one batched dma_gather per table per pass
  (amortizes the ~1us SWDGE fixed cost), vectorized attention math, and
  per-bucket one-hot PE matmuls (bf16) accumulating [msg|exp] into PSUM.
- Canonical pair-of-buckets schedule: buckets paired (2P, 2P+1); pair P gets
  ceil(max-over-cores count/128) chunks (identical across cores so the SPMD
  program is uniform; derived from the actual data at build time).
- ReduceScatter (f32) -> packed normalize (128-partition layout) -> node-major
  bf16 store -> single DMA-transpose -> W_ih matmul -> AllGather of gx.
- LSTM fixed point (ITERS passes) with software-pipelined emission: DVE runs
  only the cell scans back-to-back; gx-add is folded into PSUM via an
  identity matmul on PE; sigma/tanh on ACT; z/h muls on DVE (bf16 2x).
- FC folded into the last LSTM iteration.
"""
import os
import numpy as np
import ml_dtypes

import concourse.bass as bass
import concourse.bacc as bacc
import concourse.tile as tile
from concourse import mybir
from concourse.bass_utils import run_bass_kernel_spmd
from concourse.masks import make_identity
from contextlib import ExitStack

dt = mybir.dt
F32 = dt.float32
BF16 = dt.bfloat16
I16 = dt.int16
AF = mybir.ActivationFunctionType
ALU = mybir.AluOpType
np_bf16 = ml_dtypes.bfloat16

T, N, F_IN = 12, 10000, 64
HEADS, C, HID = 4, 32, 32
E, TN = 1_000_000, 120_000
NCORES = 8
NSH = TN // NCORES              # 15000 src-shard nodes per core
NBLK = 118                      # main node-table blocks (118*128 = 15104)
NSHP = NBLK * 128
D0 = (T - 1) * N
DN = N
DNP = 10240                     # padded live nodes
DBLK = DNP // 128               # 80 buckets
NPAIR = DBLK // 2               # 40 bucket pairs
OWN = DNP // NCORES             # 1280 owned live nodes per core
SBLK = OWN // 128               # 10 self-table blocks
NROWS = NSHP + OWN              # 16384 rows in h_tbl
HW_ROW = 256                    # h_tbl row width (bf16; 512B, %256)
AD_ROW = 128                    # ad64_tbl row width (bf16; 256B)
NPASS = 4
EAW = 7816                      # full-edge_attr wrap columns (1M vals)
LEAK = 0.2
ITERS = 4
NT = DNP
SC = 2048
NSC = NT // SC

_CACHE = {}


def _wrap16(idx):
    """int16 gather-index layout: j -> [j%16, j//16], tiled to 128 rows."""
    idx = np.asarray(idx)
    assert len(idx) % 16 == 0
    out = np.zeros((16, len(idx) // 16), np.int16)
    j = np.arange(len(idx))
    out[j % 16, j // 16] = idx.astype(np.int16)
    return np.tile(out, (8, 1))


def _chunkify(vals, cap, fill, dtype=np.float32):
    out = np.full(cap, fill, dtype)
    out[:len(vals)] = vals
    return np.ascontiguousarray(out.reshape(cap // 128, 128).T)


def _row_main(j):
    return (j % 128) * NBLK + j // 128


def _row_self(c):
    return NSHP + (c % 128) * SBLK + c // 128


def _schedule(inputs):
    """Canonical (SPMD-uniform) pair schedule from the actual edge data."""
    src = np.asarray(inputs["edge_index"][0], np.int64)
    dst = np.asarray(inputs["edge_index"][1], np.int64)
    live = (dst >= D0) & (dst < D0 + N)
    sl, dl = src[live], dst[live] - D0
    core_of = sl // NSH
    cnt = np.zeros((NCORES, NPAIR), np.int64)
    np.add.at(cnt, (core_of, ((dl % OWN) // 16) // 2), 1)
    ms = np.arange(N)
    np.add.at(cnt, (ms // OWN, ((ms % OWN) // 16) // 2), 1)
    nP = np.maximum(np.ceil(cnt.max(0) / 128).astype(int), 2)
    cstart = np.concatenate([[0], np.cumsum(nP)])   # chunk offset per pair
    nch = int(cstart[-1])
    # pass boundaries: split pairs into NPASS groups of NPAIR//NPASS
    ppp = NPAIR // NPASS
    pass_pairs = [(p * ppp, (p + 1) * ppp) for p in range(NPASS)]
    pass_chunks = [(int(cstart[a]), int(cstart[b])) for a, b in pass_pairs]
    pair_of_chunk = []
    for P in range(NPAIR):
        pair_of_chunk += [P] * int(nP[P])
    return {"nP": nP.tolist(), "cstart": cstart.tolist(), "NCH": nch,
            "pass_pairs": pass_pairs, "pass_chunks": pass_chunks,
            "pair_of_chunk": pair_of_chunk,
            "live": live, "sl": sl, "dl": dl, "core_of": core_of}


def _prep_host(inputs, sched):
    x = np.ascontiguousarray(np.asarray(inputs["x_seq"], np.float32).reshape(TN, F_IN))
    ea = np.asarray(inputs["edge_attr"], np.float32)[:, 0]
    W_gat = np.asarray(inputs["W_gat"], np.float32)
    att_src = np.asarray(inputs["att_src"], np.float32)
    att_dst = np.asarray(inputs["att_dst"], np.float32)
    att_edge = np.asarray(inputs["att_edge"], np.float32)
    W_edge = np.asarray(inputs["W_edge"], np.float32)
    gat_bias = np.asarray(inputs["gat_bias"], np.float32)
    W_ih = np.asarray(inputs["W_ih"], np.float32)
    W_hh = np.asarray(inputs["W_hh"], np.float32)
    b = np.asarray(inputs["b_ih"], np.float32) + np.asarray(inputs["b_hh"], np.float32)
    W_fc = np.asarray(inputs["W_fc"], np.float32)
    b_fc = np.asarray(inputs["b_fc"], np.float32)

    A_src = np.zeros((HEADS * C, HEADS), np.float32)
    A_dst = np.zeros((HEADS * C, HEADS), np.float32)
    for h in range(HEADS):
        A_src[h * C:(h + 1) * C, h] = att_src[h]
        A_dst[h * C:(h + 1) * C, h] = att_dst[h]
    Wa8 = np.concatenate([W_gat @ A_src, W_gat @ A_dst], axis=1)
    kap = np.array([np.dot(W_edge[0, h * C:(h + 1) * C], att_edge[h])
                    for h in range(HEADS)], np.float32)
    kap_rep = np.broadcast_to(kap, (128, HEADS)).astype(np_bf16).copy()
    gbias_rep = np.broadcast_to(gat_bias, (128, HEADS * C)).copy()
    iota128 = np.broadcast_to(np.arange(128, dtype=np.float32),
                              (128, 128)).astype(np_bf16).copy()
    c128 = np.full((128, 1), 128.0, np_bf16)
    pcol = np.arange(128, dtype=np.float32)[:, None]
    iotaP2 = np.concatenate([np.broadcast_to(pcol, (128, 128)),
                             np.broadcast_to(pcol + 128.0, (128, 128))],
                            axis=1).astype(np_bf16).copy()
    ones1 = np.ones((1, 128), np_bf16)
    perm = np.concatenate([np.arange(32, 64), np.arange(0, 32),
                           np.arange(96, 128), np.arange(64, 96)])
    WihT = np.ascontiguousarray(W_ih[perm].T.astype(np_bf16))
    WhhT = np.ascontiguousarray(W_hh[perm].T.astype(np_bf16))
    br = np.ascontiguousarray(b[perm].reshape(128, 1))
    xTD_f = x[D0:D0 + N].T                           # [64, 10000]
    xTD = np.zeros((F_IN, DNP), np_bf16)
    xTD[:, :N] = xTD_f.astype(np_bf16)

    nP, cstart, NCH = sched["nP"], sched["cstart"], sched["NCH"]
    CAP = NCH * 128
    sl, dl, core_of, live = sched["sl"], sched["dl"], sched["core_of"], sched["live"]
    ea_live = ea[live]
    ea_all = ea.reshape(NCORES, E // NCORES)

    in_maps = []
    for k in range(NCORES):
        m = core_of == k
        sLk = sl[m] - k * NSH                        # shard-local src
        dLk = dl[m]
        eak = ea_live[m]
        own_lo = OWN * k
        selfn = np.arange(own_lo, min(own_lo + OWN, N))
        # combined edge list: regular then self
        e_hs = _row_main(sLk)
        s_hs = _row_self(selfn - own_lo)
        allm = np.concatenate([dLk, selfn])
        hsrow = np.concatenate([e_hs, s_hs])
        eav = np.concatenate([eak, np.zeros(len(selfn), np.float32)])
        selfm = np.concatenate([np.zeros(len(dLk), np.float32),
                                np.ones(len(selfn), np.float32)])
        p_of = 16 * (allm // OWN) + (allm % 16)
        b_of = (allm % OWN) // 16
        adrow = p_of * DBLK + b_of
        P_of = b_of // 2
        sv = 128 * (b_of % 2) + p_of
        # pack per pair
        order = np.argsort(P_of, kind="stable")
        hs_idx = np.zeros(CAP, np.int64)
        ad_idx = np.zeros(CAP, np.int64)
        eac = np.zeros(CAP, np.float32)
        svf = np.full(CAP, -1.0, np.float32)
        smf = np.zeros(CAP, np.float32)
        Psort = P_of[order]
        bounds = np.searchsorted(Psort, np.arange(NPAIR + 1))
        for P in range(NPAIR):
            sel = order[bounds[P]:bounds[P + 1]]
            nb = len(sel)
            capP = nP[P] * 128
            assert nb <= capP, f"core {k} pair {P}: {nb} > {capP}"
            o = cstart[P] * 128
            hs_idx[o:o + nb] = hsrow[sel]
            ad_idx[o:o + nb] = adrow[sel]
            eac[o:o + nb] = eav[sel]
            svf[o:o + nb] = sv[sel]
            smf[o:o + nb] = selfm[sel]
        # per-pass wrapped int16 gather indices
        hsI = np.concatenate(
            [_wrap16(hs_idx[c0 * 128:c1 * 128]) for c0, c1 in sched["pass_chunks"]],
            axis=1)
        svT = np.ascontiguousarray(svf.reshape(1, CAP).astype(np_bf16))
        xT = np.zeros((F_IN, NSHP), np_bf16)
        xT[:, :NSH] = x[k * NSH:(k + 1) * NSH].T.astype(np_bf16)
        # xTD columns permuted to (b, p) slot order for the a_dst table matmuls
        # column b*128 + p holds node m = OWN*(p//16) + 16*b + p%16
        pq = np.arange(128)
        cols = np.empty((DBLK, 128), np.int64)
        for bb in range(DBLK):
            cols[bb] = OWN * (pq // 16) + 16 * bb + (pq % 16)
        xTDP = np.ascontiguousarray(xTD[:, cols.ravel()])
        xTDS = np.ascontiguousarray(xTD[:, own_lo:own_lo + OWN])
        eaF = np.zeros((128, 980), np.float32)
        ch = ea_all[k]
        j2 = np.arange(len(ch))
        eaF[j2 % 128, j2 // 128] = ch
        in_maps.append({
            "xT": xT, "xTDP": xTDP, "xTDS": xTDS,
            "eaC": _chunkify(eac, CAP, 0.0, np_bf16),
            "svF": _chunkify(svf, CAP, -1.0, np_bf16),
            "svH": _chunkify(svf - 128.0, CAP, -129.0, np_bf16),
            "selfM": _chunkify(smf, CAP, 0.0, np_bf16),
            "eaF": eaF, "hsI": hsI, "svT": svT,
            "Wgat": W_gat.astype(np_bf16), "Wa8": Wa8.astype(np_bf16),
            "kap": kap_rep, "gbias": gbias_rep, "iota": iota128, "c128": c128,
            "iotaP2": iotaP2, "ones1": ones1,
            "Wih": WihT, "Whh": WhhT, "br": br,
            "Wfc": np.ascontiguousarray(W_fc.reshape(HID, 1).astype(np_bf16)),
            "bfc": np.ascontiguousarray(b_fc.reshape(1, 1)),
        })
    return in_maps


def _build_nc(sched, debug=False):
    STAGE = int(os.environ.get("KSTAGE", "99"))
    nP, cstart, NCH = sched["nP"], sched["cstart"], sched["NCH"]
    pass_pairs, pass_chunks = sched["pass_pairs"], sched["pass_chunks"]
    pair_of_chunk = sched["pair_of_chunk"]
    nc = bacc.Bacc("TRN2", target_bir_lowering=False, debug=False,
                   num_devices=NCORES, num_swdge_queues=2)
    g = lambda n, s, d=F32: nc.dram_tensor(n, s, d, kind="ExternalInput").ap()
    xT = g("xT", [F_IN, NSHP], BF16)
    xTDP = g("xTDP", [F_IN, DNP], BF16)
    xTDS = g("xTDS", [F_IN, OWN], BF16)
    eaC = g("eaC", [128, NCH], BF16); svF = g("svF", [128, NCH], BF16)
    svH = g("svH", [128, NCH], BF16)
    selfM = g("selfM", [128, NCH], BF16); eaF = g("eaF", [128, 980])
    hsI = g("hsI", [128, NCH * 8], I16); svT = g("svT", [1, NCH * 128], BF16)
    Wgat = g("Wgat", [F_IN, 128], BF16); Wa8 = g("Wa8", [F_IN, 8], BF16)
    kap = g("kap", [128, HEADS], BF16); gbias = g("gbias", [128, 128])
    iota = g("iota", [128, 128], BF16); c128 = g("c128", [128, 1], BF16)
    iotaP2 = g("iotaP2", [128, 256], BF16); ones1 = g("ones1", [1, 128], BF16)
    Wih = g("Wih", [128, 128], BF16); Whh = g("Whh", [HID, 128], BF16)
    br = g("br", [128, 1]); Wfc = g("Wfc", [HID, 1], BF16); bfc = g("bfc", [1, 1])
    out = nc.dram_tensor("out", [1, NT], F32, kind="ExternalOutput").ap()
    if debug:
        dbg_gat = nc.dram_tensor("dbg_gat", [128, DNP], F32, kind="ExternalOutput").ap()
        dbg_acc = nc.dram_tensor("dbg_acc", [128, DBLK * 132], F32,
                                 kind="ExternalOutput").ap()
        dbg_rs = nc.dram_tensor("dbg_rs", [16, DBLK * 132], F32,
                                kind="ExternalOutput").ap()
        dbg_nd = nc.dram_tensor("dbg_nd", [OWN, 132], F32,
                                kind="ExternalOutput").ap()
        dbg_np = nc.dram_tensor("dbg_np", [128, (DBLK // 8) * 132], F32,
                                kind="ExternalOutput").ap()
        dbg_gx = nc.dram_tensor("dbg_gx", [128, NT], F32, kind="ExternalOutput").ap()
        dbg_h = nc.dram_tensor("dbg_h", [HID, NT], F32, kind="ExternalOutput").ap()

    h_tbl = nc.dram_tensor("h_tbl", [NROWS, HW_ROW], BF16).ap()
    acc_tbl = nc.dram_tensor("acc_tbl", [128, DBLK * 132], F32).ap()
    rs_out = nc.dram_tensor("rs_out", [16, DBLK * 132], F32).ap()
    gat_nd = nc.dram_tensor("gat_nd", [OWN, 132], BF16).ap()
    gx_sh = nc.dram_tensor("gx_sh", [128, OWN], BF16).ap()
    gx_full = nc.dram_tensor("gx_full", [NCORES, 128, OWN], BF16,
                             addr_space="Shared").ap()
    ea_in = nc.dram_tensor("ea_in", [128, 1], F32).ap()
    ea_out = nc.dram_tensor("ea_out", [128, 1], F32, addr_space="Shared").ap()
    RG = [list(range(NCORES))]

    def ecopy(e, dst, src):
        if e is nc.scalar:
            e.copy(dst, src)
        else:
            e.tensor_copy(dst, src)

    with tile.TileContext(nc) as tc, ExitStack() as top:
        const = top.enter_context(tc.tile_pool(name="const", bufs=1))
        ident = const.tile([128, 128], F32)
        make_identity(nc, ident[:])
        identb = const.tile([128, 128], BF16)
        nc.vector.tensor_copy(identb[:], ident[:])
        wgat_t = const.tile([F_IN, 128], BF16); nc.sync.dma_start(wgat_t[:], Wgat[:])
        wa8_t = const.tile([F_IN, 8], BF16); nc.sync.dma_start(wa8_t[:], Wa8[:])
        kap_t = const.tile([128, HEADS], BF16); nc.sync.dma_start(kap_t[:], kap[:])
        gb_t = const.tile([128, 128], F32); nc.sync.dma_start(gb_t[:], gbias[:])
        iota_t = const.tile([128, 128], BF16); nc.sync.dma_start(iota_t[:], iota[:])
        c128_t = const.tile([128, 1], BF16); nc.sync.dma_start(c128_t[:], c128[:])
        whh_t = const.tile([HID, 128], BF16); nc.sync.dma_start(whh_t[:], Whh[:])
        wih_t = const.tile([128, 128], BF16); nc.sync.dma_start(wih_t[:], Wih[:])
        br_t = const.tile([128, 1], F32); nc.sync.dma_start(br_t[:], br[:])
        wfc_t = const.tile([HID, 1], BF16); nc.sync.dma_start(wfc_t[:], Wfc[:])
        bfc_t = const.tile([1, 1], F32); nc.sync.dma_start(bfc_t[:], bfc[:])
        iop2_t = const.tile([128, 256], BF16); nc.sync.dma_start(iop2_t[:], iotaP2[:])
        on1_t = const.tile([1, 128], BF16); nc.sync.dma_start(on1_t[:], ones1[:])
        meanr = const.tile([128, 1], F32)
        adt_t = const.tile([128, DBLK * 4], BF16)
        sched_p = top.enter_context(tc.tile_pool(name="sched", bufs=1))
        eaC_t = sched_p.tile([128, NCH], BF16)
        nc.sync.dma_start(eaC_t[:], eaC[:])
        svF_t = sched_p.tile([128, NCH], BF16)
        nc.sync.dma_start(svF_t[:], svF[:])
        svH_t = sched_p.tile([128, NCH], BF16)
        nc.sync.dma_start(svH_t[:], svH[:])
        sm_t = sched_p.tile([128, NCH], BF16)
        nc.sync.dma_start(sm_t[:], selfM[:])
        hsI_t = sched_p.tile([128, NCH * 8], I16)
        nc.sync.dma_start(hsI_t[:], hsI[:])

        # ---------- head: load x slices up-front, A1 mean, node tables ------
        with ExitStack() as ph:
            sbh = ph.enter_context(tc.tile_pool(name="sbh", bufs=1))
            psa = ph.enter_context(tc.tile_pool(name="psa", bufs=3, space="PSUM"))
            psp = ph.enter_context(tc.tile_pool(name="psp", bufs=1, space="PSUM"))
            xt_t = sbh.tile([F_IN, NSHP], BF16)
            nc.sync.dma_start(xt_t[:], xT[:])
            xtd_t = sbh.tile([F_IN, DNP], BF16)
            nc.sync.dma_start(xtd_t[:], xTDP[:])
            xts_t = sbh.tile([F_IN, OWN], BF16)
            nc.sync.dma_start(xts_t[:], xTDS[:])

            # A1: mean(edge_attr) via AllReduce + PE broadcast; the early
            # collective doubles as a launch-skew barrier so the later
            # ReduceScatter doesn't absorb cross-core skew on the critical path
            with ExitStack() as p1:
                sbm = p1.enter_context(tc.tile_pool(name="sbm", bufs=1))
                psm = p1.enter_context(tc.tile_pool(name="psm", bufs=1,
                                                    space="PSUM"))
                eaf_t = sbm.tile([128, 980], F32)
                nc.sync.dma_start(eaf_t[:], eaF[:])
                eap = sbm.tile([128, 1], F32)
                nc.vector.tensor_reduce(eap[:], eaf_t[:], mybir.AxisListType.X,
                                        ALU.add)
                nc.sync.dma_start(ea_in[:], eap[:])
                nc.gpsimd.collective_compute("AllReduce", ALU.add,
                                             replica_groups=RG,
                                             ins=[ea_in[:]], outs=[ea_out[:]])
                eao_t = sbm.tile([128, 1], F32)
                nc.sync.dma_start(eao_t[:], ea_out[:])
                onc = sbm.tile([128, 1], F32)
                nc.gpsimd.memset(onc[:], 1.0)
                ps1 = psm.tile([1, 1], F32, space="PSUM", tag="ps1")
                nc.tensor.matmul(ps1[:], lhsT=eao_t[:], rhs=onc[:], start=True,
                                 stop=True)
                eas = sbm.tile([1, 1], F32)
                nc.scalar.mul(eas[:], ps1[:], 1.0 / E)
                onr = sbm.tile([1, 128], F32)
                nc.gpsimd.memset(onr[:], 1.0)
                ps2 = psm.tile([128, 1], F32, space="PSUM", tag="ps2")
                nc.tensor.matmul(ps2[:], lhsT=onr[:], rhs=eas[:], start=True,
                                 stop=True)
                nc.vector.tensor_copy(meanr[:], ps2[:])

            if STAGE >= 1:
                # A3: a_dst values resident in SBUF [p, b*4+h]
                pack_d = psp.tile([128, DBLK * 4], F32, space="PSUM", tag="pkd")
                for bb in range(DBLK):
                    nc.tensor.matmul(pack_d[:, bb * 4:(bb + 1) * 4],
                                     lhsT=xtd_t[:, bb * 128:(bb + 1) * 128],
                                     rhs=wa8_t[:, 4:8], start=True, stop=True)
                nc.vector.tensor_copy(adt_t[:], pack_d[:])

                # A2: main h table (118 blocks)
                HS = sbh.tile([128, NBLK * HW_ROW], BF16)
                HSv = HS[:].rearrange("p (j w) -> p j w", w=HW_ROW)
                nc.gpsimd.memset(HSv[:, :, 132:HW_ROW], 0.0)
                pack_a = psp.tile([128, NBLK * 4], F32, space="PSUM", tag="pka")
                eng = [nc.vector, nc.scalar]
                for j in range(NBLK):
                    ph_ = psa.tile([128, 128], F32, space="PSUM", tag="ph")
                    nc.tensor.matmul(ph_[:], lhsT=xt_t[:, j * 128:(j + 1) * 128],
                                     rhs=wgat_t[:], start=True, stop=True)
                    ecopy(eng[j % 2], HSv[:, j, 0:128], ph_[:])
                    nc.tensor.matmul(pack_a[:, j * 4:(j + 1) * 4],
                                     lhsT=xt_t[:, j * 128:(j + 1) * 128],
                                     rhs=wa8_t[:, 0:4], start=True, stop=True)
                nc.vector.tensor_copy(HSv[:, :, 128:132],
                                      pack_a[:].rearrange("p (j w) -> p j w", w=4))
                # store in 4 groups so DMA overlaps the matmul tail
                h_main = h_tbl[0:NSHP, :].rearrange("(p j) w -> p (j w)", p=128)
                gs = NBLK // 4 + 1
                for gi in range(4):
                    lo = gi * gs * HW_ROW
                    hi = min((gi + 1) * gs, NBLK) * HW_ROW
                    if lo < hi:
                        nc.sync.dma_start(h_main[:, lo:hi], HS[:, lo:hi])

                # A2b: own-live self rows
                HS2 = sbh.tile([128, SBLK * HW_ROW], BF16)
                HS2v = HS2[:].rearrange("p (j w) -> p j w", w=HW_ROW)
                nc.gpsimd.memset(HS2v[:, :, 132:HW_ROW], 0.0)
                pack_s = psp.tile([128, SBLK * 4], F32, space="PSUM", tag="pks")
                for j in range(SBLK):
                    ph2 = psa.tile([128, 128], F32, space="PSUM", tag="ph")
                    nc.tensor.matmul(ph2[:], lhsT=xts_t[:, j * 128:(j + 1) * 128],
                                     rhs=wgat_t[:], start=True, stop=True)
                    ecopy(eng[j % 2], HS2v[:, j, 0:128], ph2[:])
                    nc.tensor.matmul(pack_s[:, j * 4:(j + 1) * 4],
                                     lhsT=xts_t[:, j * 128:(j + 1) * 128],
                                     rhs=wa8_t[:, 0:4], start=True, stop=True)
                nc.vector.tensor_copy(HS2v[:, :, 128:132],
                                      pack_s[:].rearrange("p (j w) -> p j w", w=4))
                nc.sync.dma_start(
                    h_tbl[NSHP:NROWS, :].rearrange("(p j) w -> p (j w)", p=128),
                    HS2[:])

        if STAGE >= 2:
            # ---------- edge phase: 4 passes, batched gathers ----------
            with ExitStack() as ph:
                sbe = ph.enter_context(tc.tile_pool(name="sbe", bufs=1))
                sbp = ph.enter_context(tc.tile_pool(name="sbp", bufs=2))
                sbq = ph.enter_context(tc.tile_pool(name="sbq", bufs=2))
                pse = ph.enter_context(tc.tile_pool(name="pse", bufs=2,
                                                    space="PSUM"))
                ACCT = sbe.tile([128, DBLK * 132], F32)
                ceng = [nc.scalar, nc.vector, nc.scalar]
                cn = 0
                psr = ph.enter_context(tc.tile_pool(name="psr", bufs=2,
                                                    space="PSUM"))
                psA = ph.enter_context(tc.tile_pool(name="psA", bufs=2,
                                                    space="PSUM"))
                for p in range(NPASS):
                    c0, c1 = pass_chunks[p]
                    PC = c1 - c0
                    NIDX = PC * 128
                    NH = sbp.tile([128, PC * HW_ROW], BF16, tag="NH")
                    NHB = sbp.tile([128, PC * 132], BF16, tag="NHB")
                    NHv = NH[:].rearrange("p (e w) -> p e w", w=HW_ROW)
                    NHBv = NHB[:].rearrange("p (e w) -> p e w", w=132)
                    nc.gpsimd.dma_gather(
                        out_ap=NHv, in_ap=h_tbl[:],
                        idxs_ap=hsI_t[:, c0 * 8:c1 * 8],
                        num_idxs=NIDX, num_idxs_reg=NIDX, elem_size=HW_ROW,
                        single_packet=False, queue_num=p % 2)
                    # a_dst per slot: replicate sv down partitions (PE), build
                    # transposed one-hots, then 4-col matmuls vs resident adt_t
                    svT_p = sbq.tile([1, PC * 128], BF16, tag="svT")
                    nc.sync.dma_start(svT_p[:], svT[0:1, c0 * 128:c1 * 128])
                    AD4ps = psA.tile([128, PC * 4], F32, space="PSUM", tag="AD4")
                    for gch in range(0, PC, 4):
                        ng = min(4, PC - gch)
                        SR = psr.tile([128, 512], F32, space="PSUM", tag="SR")
                        nc.tensor.matmul(SR[:, 0:ng * 128], lhsT=on1_t[:],
                                         rhs=svT_p[:, gch * 128:(gch + ng) * 128],
                                         start=True, stop=True)
                        OT = sbq.tile([128, 1024], BF16, tag="OT")
                        OTv = OT[:].rearrange("p (l e w) -> p l e w", l=2, w=128)
                        srv = SR[:, 0:ng * 128] \
                            .rearrange("p (o e w) -> p o e w", o=1, w=128) \
                            .to_broadcast([128, 2, ng, 128])
                        iov = iop2_t[:].rearrange("p (l o w) -> p l o w",
                                                  l=2, o=1) \
                            .to_broadcast([128, 2, ng, 128])
                        nc.vector.tensor_tensor(out=OTv[:, :, 0:ng, :], in0=srv,
                                                in1=iov, op=ALU.is_equal)
                        for j in range(ng):
                            lc = gch + j
                            Pj = pair_of_chunk[c0 + lc]
                            nc.tensor.matmul(
                                AD4ps[:, lc * 4:(lc + 1) * 4],
                                lhsT=OTv[:, 0, j, :],
                                rhs=adt_t[:, (2 * Pj) * 4:(2 * Pj + 1) * 4],
                                start=True, stop=False)
                            nc.tensor.matmul(
                                AD4ps[:, lc * 4:(lc + 1) * 4],
                                lhsT=OTv[:, 1, j, :],
                                rhs=adt_t[:, (2 * Pj + 1) * 4:(2 * Pj + 2) * 4],
                                start=False, stop=True)
                    AD4t = sbq.tile([128, PC * 4], BF16, tag="AD4t")
                    nc.vector.tensor_copy(AD4t[:], AD4ps[:])
                    # q = a_src + a_dst + (ea + selfM*mean) * kap ; leaky; exp
                    EAm = sbq.tile([128, PC], F32, tag="EAm")
                    nc.vector.scalar_tensor_tensor(
                        out=EAm[:], in0=sm_t[:, c0:c1], scalar=meanr[:],
                        op0=ALU.mult, op1=ALU.add, in1=eaC_t[:, c0:c1])
                    Q4 = sbq.tile([128, PC * 4], F32, tag="Q4")
                    Q4v = Q4[:].rearrange("p (e w) -> p e w", w=4)
                    nc.vector.tensor_tensor(out=Q4v, in0=NHv[:, :, 128:132],
                                            in1=AD4t[:].rearrange(
                                                "p (e w) -> p e w", w=4),
                                            op=ALU.add)
                    T2 = sbq.tile([128, PC * 4], F32, tag="T2")
                    T2v = T2[:].rearrange("p (e w) -> p e w", w=4)
                    ea3 = EAm[:].rearrange("p (e w) -> p e w", w=1) \
                        .to_broadcast([128, PC, 4])
                    kap3 = kap_t[:].rearrange("p (o w) -> p o w", o=1) \
                        .to_broadcast([128, PC, 4])
                    nc.vector.tensor_tensor(out=T2v, in0=ea3, in1=kap3, op=ALU.mult)
                    nc.vector.tensor_tensor(out=Q4v, in0=Q4v, in1=T2v, op=ALU.add)
                    nc.vector.tensor_scalar_mul(T2[:], Q4[:], LEAK)
                    nc.vector.tensor_tensor(out=Q4[:], in0=Q4[:], in1=T2[:],
                                            op=ALU.max)
                    # exp(q) = sigmoid(q)/sigmoid(-q)
                    SG1 = sbq.tile([128, PC * 4], F32, tag="SG1")
                    nc.scalar.activation(SG1[:], Q4[:], AF.Sigmoid)
                    S4 = sbq.tile([128, PC * 4], F32, tag="S4")
                    nc.scalar.activation(S4[:], Q4[:], AF.Sigmoid, scale=-1.0)
                    nc.vector.reciprocal(S4[:], S4[:])
                    nc.vector.tensor_tensor(out=S4[:], in0=SG1[:], in1=S4[:],
                                            op=ALU.mult)
                    E4b = sbq.tile([128, PC * 4], BF16, tag="E4b")
                    nc.scalar.copy(E4b[:], S4[:])
                    S4v = S4[:].rearrange("p (e w) -> p e w", w=4)
                    nc.scalar.copy(NHBv[:, :, 128:132], S4v)
                    # expand exp to full width (bf16), then packed 2x multiply
                    E4X = sbp.tile([128, PC * 128], BF16, tag="E4X")
                    e4xv = E4X[:].rearrange("p (e h c) -> p e h c", h=HEADS, c=C)
                    e4b = E4b[:].rearrange("p (e h c) -> p e h c", h=HEADS, c=1) \
                        .to_broadcast([128, PC, HEADS, C])
                    nc.scalar.copy(e4xv, e4b)
                    nc.vector.tensor_tensor(
                        out=NHBv[:, :, 0:128], in0=NHv[:, :, 0:128],
                        in1=E4X[:].rearrange("p (e w) -> p e w", w=128),
                        op=ALU.mult)
                    # one-hots for the whole pass (bf16, 2x DVE)
                    OHL = sbp.tile([128, PC * 128], BF16, tag="OHL")
                    OHH = sbp.tile([128, PC * 128], BF16, tag="OHH")
                    OHLv = OHL[:].rearrange("p (e w) -> p e w", w=128)
                    OHHv = OHH[:].rearrange("p (e w) -> p e w", w=128)
                    sv3 = svF_t[:, c0:c1].rearrange("p (e w) -> p e w", w=1) \
                        .to_broadcast([128, PC, 128])
                    io3 = iota_t[:].rearrange("p (o w) -> p o w", o=1) \
                        .to_broadcast([128, PC, 128])
                    nc.vector.tensor_tensor(out=OHLv, in0=sv3, in1=io3,
                                            op=ALU.is_equal)
                    svh3 = svH_t[:, c0:c1].rearrange("p (e w) -> p e w", w=1) \
                        .to_broadcast([128, PC, 128])
                    nc.vector.tensor_tensor(out=OHHv, in0=svh3, in1=io3,
                                            op=ALU.is_equal)
                    for P in range(*pass_pairs[p]):
                        npc = nP[P]
                        lc = cstart[P] - c0
                        plo = pse.tile([128, 132], F32, space="PSUM", tag="plo")
                        phi = pse.tile([128, 132], F32, space="PSUM", tag="phi")
                        for j in range(npc):
                            nc.tensor.matmul(plo[:], lhsT=OHLv[:, lc + j, :],
                                             rhs=NHBv[:, lc + j, 0:132],
                                             start=(j == 0), stop=(j == npc - 1))
                            nc.tensor.matmul(phi[:], lhsT=OHHv[:, lc + j, :],
                                             rhs=NHBv[:, lc + j, 0:132],
                                             start=(j == 0), stop=(j == npc - 1))
                        ecopy(ceng[cn % 3],
                              ACCT[:, (2 * P) * 132:(2 * P + 1) * 132], plo[:])
                        cn += 1
                        ecopy(ceng[cn % 3],
                              ACCT[:, (2 * P + 1) * 132:(2 * P + 2) * 132], phi[:])
                        cn += 1
                if STAGE >= 3:
                    nc.sync.dma_start(acc_tbl[:], ACCT[:])
                    nc.gpsimd.collective_compute(
                        "ReduceScatter", ALU.add, replica_groups=RG,
                        ins=[acc_tbl[:]], outs=[rs_out[:]])
                if debug:
                    nc.sync.dma_start(dbg_acc[:], ACCT[:])

        if STAGE >= 3:
            # ---------- packed normalize + transpose --------
            with ExitStack() as ph:
                sbn = ph.enter_context(tc.tile_pool(name="sbn", bufs=1))
                NP_ = sbn.tile([128, (DBLK // 8) * 132], F32)
                # packed load: partition p' = 16*jg + p, jg = j // 10
                RW = (DBLK // 8) * 132
                for jg in range(8):
                    nc.sync.dma_start(NP_[16 * jg:16 * (jg + 1), :],
                                      rs_out[:, jg * RW:(jg + 1) * RW])
                if debug:
                    nc.sync.dma_start(dbg_np[:], NP_[:])
                JW = DBLK // 8           # 10 buckets per partition group
                NPv = NP_[:].rearrange("p (j w) -> p j w", w=132)
                nc.vector.tensor_scalar_add(NPv[:, :, 128:132],
                                            NPv[:, :, 128:132], 1e-16)
                RC = sbn.tile([128, JW * 4], F32)
                RCv = RC[:].rearrange("p (j w) -> p j w", w=4)
                nc.vector.reciprocal(RCv, NPv[:, :, 128:132])
                r4 = RCv.rearrange("p j (h c) -> p j h c", c=1) \
                    .to_broadcast([128, JW, HEADS, C])
                m4 = NPv[:, :, 0:128].rearrange("p j (h c) -> p j h c", h=HEADS)
                nc.vector.tensor_tensor(out=m4, in0=m4, in1=r4, op=ALU.mult)
                gbb = gb_t[:].rearrange("p (o w) -> p o w", o=1) \
                    .to_broadcast([128, JW, 128])
                nc.vector.tensor_tensor(out=NPv[:, :, 0:128], in0=NPv[:, :, 0:128],
                                        in1=gbb, op=ALU.add)
                nc.vector.tensor_scalar_max(NPv[:, :, 0:128], NPv[:, :, 0:128], 0.0)
                NB = sbn.tile([128, JW * 132], BF16)
                nc.vector.tensor_copy(NB[:], NP_[:])
                # node-major store: node = 160*jg + 16*jj + p (8 3D DMAs)
                for jg in range(8):
                    ndv = gat_nd[160 * jg:160 * (jg + 1), :] \
                        .rearrange("(jj p) w -> p jj w", p=16)
                    nbv = NB[16 * jg:16 * (jg + 1), :] \
                        .rearrange("p (jj w) -> p jj w", w=132)
                    nc.sync.dma_start(ndv, nbv)
                if debug:
                    nc.sync.dma_start(dbg_rs[:], rs_out[:])
                    nc.gpsimd.dma_start(dbg_nd[:], gat_nd[:])
        if STAGE >= 4:
            # ---------- transpose-load + W_ih matmul + AllGather ----------
            with ExitStack() as ph:
                sbg = ph.enter_context(tc.tile_pool(name="sbg", bufs=1))
                psg = ph.enter_context(tc.tile_pool(name="psg", bufs=3,
                                                    space="PSUM"))
                GBt = sbg.tile([128, OWN], BF16)
                nc.sync.dma_start_transpose(GBt[:], gat_nd[:, 0:128])
                gxs = sbg.tile([128, OWN], BF16)
                for q in range(OWN // 512 + (1 if OWN % 512 else 0)):
                    lo = q * 512
                    hi = min(lo + 512, OWN)
                    pg = psg.tile([128, 512], F32, space="PSUM", tag="pg")
                    nc.tensor.matmul(pg[:, 0:hi - lo], lhsT=wih_t[:],
                                     rhs=GBt[:, lo:hi], start=True, stop=True)
                    nc.vector.tensor_scalar_add(gxs[:, lo:hi], pg[:, 0:hi - lo],
                                                br_t[:])
                nc.sync.dma_start(gx_sh[:], gxs[:])
            nc.gpsimd.collective_compute("AllGather", ALU.bypass,
                                         replica_groups=RG,
                                         ins=[gx_sh[:]], outs=[gx_full[:]])

        # ---------- LSTM fixed point (software-pipelined emission) ----------
        persist = top.enter_context(tc.tile_pool(name="persist", bufs=1))
        gxt = persist.tile([128, NT], BF16)
        H = persist.tile([HID, NT + 32], BF16)
        nc.gpsimd.memset(H[:], 0.0)
        if STAGE >= 5:
            nc.sync.dma_start(gxt[:].rearrange("p (k n) -> p k n", k=NCORES),
                              gx_full[:].rearrange("k p n -> p k n"))
        else:
            nc.gpsimd.memset(gxt[:], 0.0)
        if debug:
            nc.gpsimd.dma_start(dbg_gx[:], gxt[:])
        if STAGE >= 6:
            with ExitStack() as ph:
                sbl = ph.enter_context(tc.tile_pool(name="sbl", bufs=7))
                sbc = ph.enter_context(tc.tile_pool(name="sbc", bufs=3))
                sbo = ph.enter_context(tc.tile_pool(name="sbo", bufs=2))
                psl = ph.enter_context(tc.tile_pool(name="psl", bufs=3,
                                                    space="PSUM"))
                psf = ph.enter_context(tc.tile_pool(name="psf", bufs=2,
                                                    space="PSUM"))
                S_t = [None] * NSC
                Tg_t = [None] * NSC
                Zt_t = [None] * NSC
                Ct_t = [None] * NSC

                def emit_zt(s):
                    Zt = sbl.tile([HID, SC], BF16, tag="Zt")
                    nc.vector.tensor_tensor(out=Zt[:], in0=S_t[s][32:64, :],
                                            in1=Tg_t[s][32:64, :], op=ALU.mult)
                    Zt_t[s] = Zt

                def stage_in(i, s):
                    """matmuls into PSUM, activations, z — for iteration i."""
                    lo = s * SC
                    S_ = sbl.tile([96, SC], BF16, tag="S")
                    Tg = sbl.tile([64, SC], BF16, tag="Tg")
                    if i == 0:
                        nc.scalar.activation(S_[:], gxt[0:96, lo:lo + SC],
                                             AF.Sigmoid)
                        nc.scalar.activation(Tg[32:64, :], gxt[96:128, lo:lo + SC],
                                             AF.Tanh)
                    else:
                        for q in range(SC // 1024):
                            a = lo + q * 1024
                            pG = psl.tile([128, 1024], F32, space="PSUM", tag="pG")
                            for hh in range(2):
                                ha, hb = a + hh * 512, a + (hh + 1) * 512
                                nc.tensor.matmul(pG[:, hh * 512:(hh + 1) * 512],
                                                 lhsT=whh_t[:], rhs=H[:, ha:hb],
                                                 start=True, stop=False)
                                nc.tensor.matmul(pG[:, hh * 512:(hh + 1) * 512],
                                                 lhsT=identb[:], rhs=gxt[:, ha:hb],
                                                 start=False, stop=True)
                            nc.scalar.activation(S_[:, q * 1024:(q + 1) * 1024],
                                                 pG[0:96, :], AF.Sigmoid)
                            nc.scalar.activation(Tg[32:64, q * 1024:(q + 1) * 1024],
                                                 pG[96:128, :], AF.Tanh)
                    S_t[s] = S_
                    Tg_t[s] = Tg

                TC_t = [None] * NSC

                def stage_out(i, s):
                    """tanh(c) — for iteration i."""
                    TC = sbo.tile([96, SC], BF16, tag="TC")
                    nc.scalar.activation(TC[64:96, :], Ct_t[s][:], AF.Tanh)
                    TC_t[s] = TC

                def emit_hm(i, s):
                    lo = s * SC
                    nc.vector.tensor_tensor(out=H[:, lo + 1:lo + SC + 1],
                                            in0=S_t[s][64:96, :],
                                            in1=TC_t[s][64:96, :], op=ALU.mult)

                for s in range(NSC):
                    stage_in(0, s)
                    emit_zt(s)
                def fc_chunk(s):
                    lo = s * SC
                    OFc = sbo.tile([1, SC], F32, tag="OFc")
                    for q in range(SC // 512):
                        pf = psf.tile([1, 512], F32, space="PSUM", tag="pf")
                        nc.tensor.matmul(
                            pf[:], lhsT=wfc_t[:],
                            rhs=H[:, 1 + lo + q * 512:1 + lo + (q + 1) * 512],
                            start=True, stop=True)
                        nc.vector.tensor_scalar_add(
                            OFc[:, q * 512:(q + 1) * 512], pf[:], bfc_t[:])
                    nc.sync.dma_start(out[:, lo:lo + SC], OFc[:])

                for i in range(ITERS):
                    for s in range(NSC):
                        Ct = sbc.tile([HID, SC], F32, tag="Ct")
                        nc.vector.tensor_tensor_scan(
                            out=Ct[:], data0=S_t[s][0:32, :], data1=Zt_t[s][:],
                            initial=(0.0 if s == 0 else Ct_t[s - 1][:, SC - 1:SC]),
                            op0=ALU.mult, op1=ALU.add)
                        Ct_t[s] = Ct
                        stage_out(i, s)
                        # lagged emissions so DVE never stalls behind the
                        # cross-iteration tc->hm->matmul->sigmoid chain:
                        # hm and the next iteration's inputs lag 1 chunk,
                        # the next iteration's z-mult lags 3 chunks
                        if s >= 1:
                            emit_hm(i, s - 1)
                            if i == ITERS - 1:
                                fc_chunk(s - 1)
                            elif i + 1 < ITERS:
                                stage_in(i + 1, s - 1)
                                if s >= 4:
                                    emit_zt(s - 4)
                    emit_hm(i, NSC - 1)
                    if i == ITERS - 1:
                        fc_chunk(NSC - 1)
                    if i + 1 < ITERS:
                        stage_in(i + 1, NSC - 1)
                        for sz in range(max(0, NSC - 4), NSC):
                            emit_zt(sz)
        else:
            with ExitStack() as ph:
                sbf = ph.enter_context(tc.tile_pool(name="sbf", bufs=1))
                OF = sbf.tile([1, NT], F32)
                nc.gpsimd.memset(OF[:], 0.0)
                nc.sync.dma_start(out[:], OF[:])
        if debug:
            nc.gpsimd.dma_start(dbg_h[:], H[:, 1:NT + 1])
            with ExitStack() as ph:
                sbd = ph.enter_context(tc.tile_pool(name="sbd", bufs=1))
                DG = sbd.tile([128, OWN], F32)
                DB = sbd.tile([128, OWN], BF16)
                nc.sync.dma_start_transpose(DB[:], gat_nd[:, 0:128])
                nc.vector.tensor_copy(DG[:], DB[:])
                nc.sync.dma_start(dbg_gat[:, 0:OWN], DG[:])

    nc.compile()
    return nc


def run(inputs, trace=False, debug=False):
    sched = _schedule(inputs)
    key = ("dbg" if debug else "rel", sched["NCH"], tuple(sched["nP"]))
    if key not in _CACHE:
        _CACHE[key] = _build_nc(sched, debug=debug)
    nc = _CACHE[key]
    in_maps = _prep_host(inputs, sched)
    res = run_bass_kernel_spmd(nc, in_maps, list(range(NCORES)), trace=trace)
    return res


def kernel(**inputs) -> np.ndarray:
    res = run(inputs)
    o = res.results[0]["out"]
    return np.ascontiguousarray(o[0, :N].reshape(N, 1).astype(np.float32))
